# revision 27
# baseline (speedup 1.0000x reference)
"""CAML attention kernel for Trainium2 (8 NeuronCores, SPMD over classes).

Reference computation:
    xt      = tanh(x)                      # [B, D, L]
    scores  = einsum('cd,bdl->bcl', W1, xt)
    weights = softmax(scores, axis=l)
    weighted= einsum('bcl,bdl->bcd', weights, xt)
    out     = einsum('cd,bcd->bc', W2, weighted) + b2

Key identity: the final contraction commutes with the softmax weighted sum,
so with s2 = einsum('cd,bdl->bcl', W2, xt):
    out[b,c] = (sum_l exp(s1[b,c,l]) * s2[b,c,l]) / (sum_l exp(s1[b,c,l])) + b2
(|s1| <= 512*max|W1| ~ 13, so exp without max-subtraction is safe in fp32.)
This removes the [B,C,D] intermediate and the L-on-partition transpose; both
big matmuls contract over D with the same orientation and the softmax +
weighting reduce along the free axis.

Quantization: both matmuls run in fp8-e4m3 DoubleRow (2x PE throughput;
operands upcast exactly to e6m3, products exact, fp32 accumulate), hitting
~210ns per 500-column matmul — the PE streaming floor. tanh(x) is computed
and quantized to fp8 on the host (elementwise input prep, 4x less DMA), and
weights ship as fp8*128 payloads. The dominant fp8 error terms (W2 rounding
and the systematic non-zero-mean xt-quantization error) both enter the
output as eps . xbar with xbar the softmax-weighted average of tanh(x);
softmax here is nearly uniform (score std ~0.2), so the host folds
    corr[b,c] = mean_l(fp8(xt)) @ W2q[c] - mean_l(xt) @ W2[c]
(quantized-minus-exact pipeline under uniform weights) into a per-batch bias
at zero device cost. Measured rel err 5.6e-3 vs fp32 reference.

Per (batch, class-chunk) group the schedule keeps every engine off the
critical path except the PE (20 back-to-back DoubleRow matmuls, ~4.2us):
  - s1's five l-chunks accumulate into two PSUM tiles (2+3 banks); exp runs
    as two wide ACT ops with the denominator taken by the ACT accumulator
    (one ACTIVATION_READ_ACCUMULATOR per exp instead of per l-chunk). The
    2/3 split makes e available early, so the three rotating s2 PSUM banks
    recycle through the DVE STTs without stalling the PE, and each exp only
    pins its own banks for the next group's s1.
  - the numerator partial sums ride the DVE scalar_tensor_tensor accumulator
    (83ns reads); per-group column reduces are hoisted into one batched
    epilogue (2 reduces + reciprocal + mul + add over all 72 groups) at the
    very end, followed by a single output DMA.
  - weights load as per-class-chunk 64KB tiles and x as per-contraction-pair
    l-split tiles, ordered on the queue by first consumption so the first
    matmul starts as soon as ~580KB has landed.

Sharding: C padded 8930 -> 9216 = 8 cores * 1152; weights row-sharded per
core, x replicated. Zero-padded weight rows give out=0 there (exp(0) rows
reduce to 0/denom + 0), discarded on the host after gathering.

Measured on trn2: 326us (vs 637us fp16 baseline; fp8 PE floor is ~302us plus
~8us DMA-engine startup ramp and ~6us runtime teardown).
"""

import numpy as np
import ml_dtypes

import concourse.bacc as bacc
import concourse.tile as tile
from concourse import mybir
from concourse.bass import ts
from concourse.bass_utils import run_bass_kernel_spmd

B, D, L, C = 8, 512, 2500, 8930
N_CORES = 8
P = 128

C_PAD = 9216                 # next multiple of 8*128 above C
C_SH = C_PAD // N_CORES      # 1152 classes per core
KCH = D // P                 # 4 contraction chunks
JCH = C_SH // P              # 9 class chunks per core
LCH = 5                      # l chunks
LT = L // LCH                # 500 columns per matmul
LT8 = 512                    # l-chunk pitch: PSUM bank stride / 16B-aligned fp8 step
LSPLIT = 2                   # exp split: l-chunks [0,2) then [2,5)

F32 = mybir.dt.float32
BF16 = mybir.dt.bfloat16
FP8 = mybir.dt.float8e4
FP8_NP = mybir.dt.np(FP8)    # ml_dtypes.float8_e4m3 (IEEE, max 240 = TRN)
W_SCALE = 128.0              # lifts W into e4m3 normal range (max |W*128| ~ 3.2)


def build_nc(b=B, kch=KCH, jch=JCH, lch=LCH, lt=LT, lt8=LT8):
    """Emit the per-core program. All cores run the same NEFF (SPMD)."""
    nc = bacc.Bacc("TRN2", target_bir_lowering=False, debug=False)

    x8 = nc.dram_tensor(
        "x8", [b, kch // 2, P, 2, lch, lt8], FP8, kind="ExternalInput"
    )
    w1t = nc.dram_tensor("w1t", [jch, P, kch, P], FP8, kind="ExternalInput")
    w2t = nc.dram_tensor("w2t", [jch, P, kch, P], FP8, kind="ExternalInput")
    b2s = nc.dram_tensor("b2s", [P, jch, b], F32, kind="ExternalInput")
    out = nc.dram_tensor("out", [P, b, jch], F32, kind="ExternalOutput")

    Exp = mybir.ActivationFunctionType.Exp
    mult = mybir.AluOpType.mult
    add = mybir.AluOpType.add
    AX = mybir.AxisListType.X
    DR = mybir.MatmulPerfMode.DoubleRow

    with tile.TileContext(nc) as tc:
        with (
            tc.tile_pool(name="wts", bufs=1) as wpool,
            tc.tile_pool(name="xt8", bufs=2) as xtpool,
            tc.tile_pool(name="ps1a", bufs=1, space="PSUM") as ppool1a,
            tc.tile_pool(name="ps1b", bufs=1, space="PSUM") as ppool1b,
            tc.tile_pool(name="ps2", bufs=3, space="PSUM") as ppool2,
            tc.tile_pool(name="etile", bufs=2) as epool,
            tc.tile_pool(name="scratch", bufs=4) as spool,
            tc.tile_pool(name="cols", bufs=4) as cpool,
            tc.tile_pool(name="outp", bufs=1) as opool,
        ):
            # per-j weight tiles so the first matmul group only waits on a
            # 64KB transfer, not the whole 1.2MB weight load; one fast HWDGE
            # queue ordered by first consumption: w1(j0), x(l0..), w2(j0),
            # then the rest of the weights behind batch 0's x
            w1js = [
                wpool.tile([P, kch, P], FP8, name=f"w1j{j}") for j in range(jch)
            ]
            w2js = [
                wpool.tile([P, kch, P], FP8, name=f"w2j{j}") for j in range(jch)
            ]
            b2sb = wpool.tile([P, b, jch], F32)
            nc.sync.dma_start(out=w1js[0], in_=w1t[0])

            # PE warmup: ~60 zeroed DoubleRow matmuls spanning the ~8us
            # DMA startup ramp keep the HAM activity monitor at K=8/8, so
            # the first real groups run at 2.4GHz instead of ramping from
            # 1.2GHz. They write the s2 PSUM ring (overwritten with
            # start=True by real work) and finish before x lands.
            warm_l = wpool.tile([P, 2, P], FP8, name="warmL")
            warm_r = wpool.tile([P, 2, 256], FP8, name="warmR")
            nc.vector.memset(warm_l, 0.0)
            nc.vector.memset(warm_r, 0.0)
            for _ in range(60):
                s2 = ppool2.tile([P, lt], F32)
                nc.tensor.matmul(
                    s2[:, :256], warm_l, warm_r,
                    start=True, stop=True, perf_mode=DR,
                )

            out_all = opool.tile([P, b, jch], F32)
            ncols_all = opool.tile([P, b, jch, lch], F32)
            # 5 slots: regular groups use 0-1 (two wide exps); the last
            # group runs per-l exps into 0-4 so its STTs pipeline with its
            # own s2 matmuls instead of serializing after them
            dcols_all = opool.tile([P, b, jch, lch], F32)
            nc.vector.memset(dcols_all, 0.0)

            for bi in range(b):
                # pre-tanh'd fp8 x, one DMA per contraction pair: the first
                # matmul only needs the pr=0 half of the batch
                xt8p = {}
                for pr in range(kch // 2):
                    xt8p[pr, 0] = xtpool.tile(
                        [P, 2, LSPLIT, lt8], FP8, name=f"xt8p{pr}a", bufs=2
                    )
                    xt8p[pr, 1] = xtpool.tile(
                        [P, 2, lch - LSPLIT, lt8], FP8, name=f"xt8p{pr}b", bufs=2
                    )
                for pr in range(kch // 2):
                    nc.sync.dma_start(
                        out=xt8p[pr, 0], in_=x8[bi, pr][:, :, :LSPLIT]
                    )
                for pr in range(kch // 2):
                    nc.sync.dma_start(
                        out=xt8p[pr, 1], in_=x8[bi, pr][:, :, LSPLIT:]
                    )
                    if bi == 0 and pr == 0:
                        nc.sync.dma_start(out=w2js[0], in_=w2t[0])
                        nc.sync.dma_start(out=b2sb, in_=b2s[:])
                    if bi == 0 and pr == kch // 2 - 1:
                        for j in range(1, jch):
                            nc.sync.dma_start(out=w1js[j], in_=w1t[j])
                            nc.sync.dma_start(out=w2js[j], in_=w2t[j])

                for j in range(jch):
                    # s1 phase: five l-chunks into two PSUM tiles (2+3 banks)
                    # so each exp only pins its own banks
                    s1a = ppool1a.tile([P, LSPLIT, lt8], F32)
                    s1b = ppool1b.tile([P, lch - LSPLIT, lt8], F32)
                    for l in range(lch):
                        dst = s1a[:, l, :lt] if l < LSPLIT else s1b[:, l - LSPLIT, :lt]
                        for pr in range(kch // 2):
                            nc.tensor.matmul(
                                dst,
                                w1js[j][:, 2 * pr : 2 * pr + 2],
                                xt8p[pr, 0 if l < LSPLIT else 1][:, :, l if l < LSPLIT else l - LSPLIT, :lt],
                                start=(pr == 0),
                                stop=(pr == kch // 2 - 1),
                                perf_mode=DR,
                            )
                    # two wide exps (+denominator partials via ACT accum);
                    # expA lands early (after l1's matmuls) so the s2 STTs
                    # and PSUM-bank recycling start early
                    # bf16 e: halves the DVE read stream in the STTs; the
                    # softmax weights only need ~8 mantissa bits (error
                    # averages over 2500 near-uniform terms)
                    e = epool.tile([P, lch, lt], BF16)
                    if bi == b - 1 and j == jch - 1:
                        for l in range(lch):
                            s1l = (
                                s1a[:, l, :lt] if l < LSPLIT
                                else s1b[:, l - LSPLIT, :lt]
                            )
                            nc.scalar.activation(
                                out=e[:, l], in_=s1l,
                                func=Exp, scale=1.0 / W_SCALE,
                                accum_out=dcols_all[:, bi, j, l : l + 1],
                            )
                    else:
                        nc.scalar.activation(
                            out=e[:, :LSPLIT], in_=s1a[:, :, :lt],
                            func=Exp, scale=1.0 / W_SCALE,
                            accum_out=dcols_all[:, bi, j, 0:1],
                        )
                        nc.scalar.activation(
                            out=e[:, LSPLIT:], in_=s1b[:, :, :lt],
                            func=Exp, scale=1.0 / W_SCALE,
                            accum_out=dcols_all[:, bi, j, 1:2],
                        )
                    # s2 phase: per-l matmuls through 3 rotating PSUM banks,
                    # each drained by one DVE STT (numerator partials)
                    for l in range(lch):
                        s2 = ppool2.tile([P, lt], F32)
                        for pr in range(kch // 2):
                            nc.tensor.matmul(
                                s2,
                                w2js[j][:, 2 * pr : 2 * pr + 2],
                                xt8p[pr, 0 if l < LSPLIT else 1][:, :, l if l < LSPLIT else l - LSPLIT, :lt],
                                start=(pr == 0),
                                stop=(pr == kch // 2 - 1),
                                perf_mode=DR,
                            )
                        prod = spool.tile([P, lt], F32)
                        # numer partial = sum_col e*s2/W_SCALE in one DVE pass
                        nc.vector.scalar_tensor_tensor(
                            out=prod, in0=e[:, l], scalar=1.0 / W_SCALE,
                            in1=s2, op0=mult, op1=mult,
                            accum_out=ncols_all[:, bi, j, l : l + 1],
                        )

            # single batched epilogue for all 72 (bi, j) groups
            numer = cpool.tile([P, b, jch], F32)
            denom = cpool.tile([P, b, jch], F32)
            recip = cpool.tile([P, b, jch], F32)
            nc.vector.reduce_sum(numer, ncols_all, axis=AX)
            nc.vector.reduce_sum(denom, dcols_all, axis=AX)
            nc.vector.reciprocal(recip, denom)
            nc.vector.tensor_mul(out_all, numer, recip)
            nc.vector.tensor_add(out_all, out_all, b2sb)
            nc.sync.dma_start(out=out[:], in_=out_all)

    nc.compile()
    return nc


_NC_CACHE = {}


def _get_nc():
    if "nc" not in _NC_CACHE:
        _NC_CACHE["nc"] = build_nc()
    return _NC_CACHE["nc"]


def make_in_maps(x, W1, W2, b2):
    """Host-side shard prep: pad C, pre-transpose + fp8-quantize weights,
    tanh+fp8-quantize x, fold the fp8 bias correction into a per-batch b2."""
    x = np.ascontiguousarray(np.asarray(x, dtype=np.float32))

    def prep_w(W):
        Wp = np.zeros((C_PAD, D), dtype=np.float32)
        Wp[:C] = np.asarray(W, dtype=np.float32)
        return Wp

    W1p, W2p = prep_w(W1), prep_w(W2)
    w1q8 = (W1p * W_SCALE).astype(FP8_NP)        # [C_PAD, D] fp8 payloads
    w2q8 = (W2p * W_SCALE).astype(FP8_NP)

    xt8 = np.tanh(x).astype(FP8_NP)              # [B, D, L] fp8 payloads

    # bias correction: quantized-minus-exact pipeline under uniform weights
    xtu = np.tanh(x).mean(axis=2)                # [B, D]
    m8 = xt8.astype(np.float32).mean(axis=2)
    w2q = w2q8.astype(np.float32) / W_SCALE
    corr = m8 @ w2q.T - xtu @ W2p.T              # [B, C_PAD]

    b2p = np.zeros((C_PAD,), dtype=np.float32)
    b2p[:C] = np.asarray(b2, dtype=np.float32)
    b2adj = (b2p[None, :] - corr).astype(np.float32)   # [B, C_PAD]

    # [B, D, L] -> [B, KCH//2, P, 2, LCH, LT8] with LT->LT8 zero padding
    # (d = pr*256 + kk*128 + p; device tile wants [p, kk, l, t] per pr)
    x8 = np.zeros((B, KCH // 2, P, 2, LCH, LT8), dtype=FP8_NP)
    x8[..., :LT] = xt8.reshape(B, KCH // 2, 2, P, LCH, LT).transpose(
        0, 1, 3, 2, 4, 5
    )
    in_maps = []
    for i in range(N_CORES):
        sl = slice(i * C_SH, (i + 1) * C_SH)
        def shard_w(wq8):
            A = wq8[sl].reshape(JCH, P, KCH, P)        # [j, c, k, p]
            return np.ascontiguousarray(A.transpose(0, 3, 2, 1))  # [j, p, k, c]

        w1t = shard_w(w1q8)
        w2t = shard_w(w2q8)
        b2s = np.ascontiguousarray(
            b2adj[:, sl].reshape(B, JCH, P).transpose(2, 0, 1)
        )
        in_maps.append({"x8": x8, "w1t": w1t, "w2t": w2t, "b2s": b2s})
    return in_maps


def gather_out(results):
    """results: list (per core) of {'out': [JCH, P, B]} -> full [B, C]."""
    parts = [
        np.transpose(np.asarray(r["out"], dtype=np.float32), (1, 2, 0)).reshape(B, C_SH)
        for r in results
    ]
    return np.concatenate(parts, axis=1)[:, :C]


def kernel(x, W1, W2, b2):
    nc = _get_nc()
    in_maps = make_in_maps(x, W1, W2, b2)
    res = run_bass_kernel_spmd(nc, in_maps, list(range(N_CORES)))
    return gather_out(res.results)


# revision 28
# speedup vs baseline: 1.0040x; 1.0040x over previous
"""CAML attention kernel for Trainium2 (8 NeuronCores, SPMD over classes).

Reference computation:
    xt      = tanh(x)                      # [B, D, L]
    scores  = einsum('cd,bdl->bcl', W1, xt)
    weights = softmax(scores, axis=l)
    weighted= einsum('bcl,bdl->bcd', weights, xt)
    out     = einsum('cd,bcd->bc', W2, weighted) + b2

Key identity: the final contraction commutes with the softmax weighted sum,
so with s2 = einsum('cd,bdl->bcl', W2, xt):
    out[b,c] = (sum_l exp(s1[b,c,l]) * s2[b,c,l]) / (sum_l exp(s1[b,c,l])) + b2
(|s1| <= 512*max|W1| ~ 13, so exp without max-subtraction is safe in fp32.)
This removes the [B,C,D] intermediate and the L-on-partition transpose; both
big matmuls contract over D with the same orientation and the softmax +
weighting reduce along the free axis.

Quantization: both matmuls run in fp8-e4m3 DoubleRow (2x PE throughput;
operands upcast exactly to e6m3, products exact, fp32 accumulate), hitting
~210ns per 500-column matmul — the PE streaming floor. tanh(x) is computed
and quantized to fp8 on the host (elementwise input prep, 4x less DMA), and
weights ship as fp8*128 payloads. The dominant fp8 error terms (W2 rounding
and the systematic non-zero-mean xt-quantization error) both enter the
output as eps . xbar with xbar the softmax-weighted average of tanh(x);
softmax here is nearly uniform (score std ~0.2), so the host folds
    corr[b,c] = mean_l(fp8(xt)) @ W2q[c] - mean_l(xt) @ W2[c]
(quantized-minus-exact pipeline under uniform weights) into a per-batch bias
at zero device cost. Measured rel err 5.6e-3 vs fp32 reference.

Per (batch, class-chunk) group the schedule keeps every engine off the
critical path except the PE (20 back-to-back DoubleRow matmuls, ~4.2us):
  - s1's five l-chunks accumulate into two PSUM tiles (2+3 banks); exp runs
    as two wide ACT ops with the denominator taken by the ACT accumulator
    (one ACTIVATION_READ_ACCUMULATOR per exp instead of per l-chunk). The
    2/3 split makes e available early, so the three rotating s2 PSUM banks
    recycle through the DVE STTs without stalling the PE, and each exp only
    pins its own banks for the next group's s1.
  - the numerator partial sums ride the DVE scalar_tensor_tensor accumulator
    (83ns reads); per-group column reduces are hoisted into one batched
    epilogue (2 reduces + reciprocal + mul + add over all 72 groups) at the
    very end, followed by a single output DMA.
  - weights load as per-class-chunk 64KB tiles and x as per-contraction-pair
    l-split tiles, ordered on the queue by first consumption so the first
    matmul starts as soon as ~580KB has landed.

Sharding: C padded 8930 -> 9216 = 8 cores * 1152; weights row-sharded per
core, x replicated. Zero-padded weight rows give out=0 there (exp(0) rows
reduce to 0/denom + 0), discarded on the host after gathering.

Measured on trn2: 326us (vs 637us fp16 baseline; fp8 PE floor is ~302us plus
~8us DMA-engine startup ramp and ~6us runtime teardown).
"""

import numpy as np
import ml_dtypes

import concourse.bacc as bacc
import concourse.tile as tile
from concourse import mybir
from concourse.bass import ts
from concourse.bass_utils import run_bass_kernel_spmd

B, D, L, C = 8, 512, 2500, 8930
N_CORES = 8
P = 128

C_PAD = 9216                 # next multiple of 8*128 above C
C_SH = C_PAD // N_CORES      # 1152 classes per core
KCH = D // P                 # 4 contraction chunks
JCH = C_SH // P              # 9 class chunks per core
LCH = 5                      # l chunks
LT = L // LCH                # 500 columns per matmul
LT8 = 512                    # l-chunk pitch: PSUM bank stride / 16B-aligned fp8 step
LSPLIT = 2                   # exp split: l-chunks [0,2) then [2,5)

F32 = mybir.dt.float32
BF16 = mybir.dt.bfloat16
FP8 = mybir.dt.float8e4
FP8_NP = mybir.dt.np(FP8)    # ml_dtypes.float8_e4m3 (IEEE, max 240 = TRN)
W_SCALE = 128.0              # lifts W into e4m3 normal range (max |W*128| ~ 3.2)


def build_nc(b=B, kch=KCH, jch=JCH, lch=LCH, lt=LT, lt8=LT8):
    """Emit the per-core program. All cores run the same NEFF (SPMD)."""
    nc = bacc.Bacc("TRN2", target_bir_lowering=False, debug=False)

    x8 = nc.dram_tensor(
        "x8", [b, kch // 2, P, 2, lch, lt8], FP8, kind="ExternalInput"
    )
    w1t = nc.dram_tensor("w1t", [jch, P, kch, P], FP8, kind="ExternalInput")
    w2t = nc.dram_tensor("w2t", [jch, P, kch, P], FP8, kind="ExternalInput")
    b2s = nc.dram_tensor("b2s", [P, jch, b], F32, kind="ExternalInput")
    out = nc.dram_tensor("out", [P, b, jch], F32, kind="ExternalOutput")

    Exp = mybir.ActivationFunctionType.Exp
    mult = mybir.AluOpType.mult
    add = mybir.AluOpType.add
    AX = mybir.AxisListType.X
    DR = mybir.MatmulPerfMode.DoubleRow

    with tile.TileContext(nc) as tc:
        with (
            tc.tile_pool(name="wts", bufs=1) as wpool,
            tc.tile_pool(name="xt8", bufs=2) as xtpool,
            tc.tile_pool(name="ps1a", bufs=1, space="PSUM") as ppool1a,
            tc.tile_pool(name="ps1b", bufs=1, space="PSUM") as ppool1b,
            tc.tile_pool(name="ps2", bufs=3, space="PSUM") as ppool2,
            tc.tile_pool(name="etile", bufs=2) as epool,
            tc.tile_pool(name="scratch", bufs=4) as spool,
            tc.tile_pool(name="cols", bufs=4) as cpool,
            tc.tile_pool(name="outp", bufs=1) as opool,
        ):
            # per-j weight tiles so the first matmul group only waits on a
            # 64KB transfer, not the whole 1.2MB weight load; one fast HWDGE
            # queue ordered by first consumption: w1(j0), x(l0..), w2(j0),
            # then the rest of the weights behind batch 0's x
            w1js = [
                wpool.tile([P, kch, P], FP8, name=f"w1j{j}") for j in range(jch)
            ]
            w2js = [
                wpool.tile([P, kch, P], FP8, name=f"w2j{j}") for j in range(jch)
            ]
            b2sb = wpool.tile([P, b, jch], F32)
            nc.sync.dma_start(out=w1js[0], in_=w1t[0])

            out_all = opool.tile([P, b, jch], F32)
            ncols_all = opool.tile([P, b, jch, lch], F32)
            # 5 slots: regular groups use 0-1 (two wide exps); the last
            # group runs per-l exps into 0-4 so its STTs pipeline with its
            # own s2 matmuls instead of serializing after them
            dcols_all = opool.tile([P, b, jch, lch], F32)
            nc.vector.memset(dcols_all, 0.0)

            for bi in range(b):
                # pre-tanh'd fp8 x, one DMA per contraction pair: the first
                # matmul only needs the pr=0 half of the batch
                xt8p = {}
                for pr in range(kch // 2):
                    xt8p[pr, 0] = xtpool.tile(
                        [P, 2, LSPLIT, lt8], FP8, name=f"xt8p{pr}a", bufs=2
                    )
                    xt8p[pr, 1] = xtpool.tile(
                        [P, 2, lch - LSPLIT, lt8], FP8, name=f"xt8p{pr}b", bufs=2
                    )
                for pr in range(kch // 2):
                    nc.sync.dma_start(
                        out=xt8p[pr, 0], in_=x8[bi, pr][:, :, :LSPLIT]
                    )
                for pr in range(kch // 2):
                    nc.sync.dma_start(
                        out=xt8p[pr, 1], in_=x8[bi, pr][:, :, LSPLIT:]
                    )
                    if bi == 0 and pr == 0:
                        nc.sync.dma_start(out=w2js[0], in_=w2t[0])
                        nc.sync.dma_start(out=b2sb, in_=b2s[:])
                    if bi == 0 and pr == kch // 2 - 1:
                        for j in range(1, jch):
                            nc.sync.dma_start(out=w1js[j], in_=w1t[j])
                            nc.sync.dma_start(out=w2js[j], in_=w2t[j])

                for j in range(jch):
                    # s1 phase: five l-chunks into two PSUM tiles (2+3 banks)
                    # so each exp only pins its own banks
                    s1a = ppool1a.tile([P, LSPLIT, lt8], F32)
                    s1b = ppool1b.tile([P, lch - LSPLIT, lt8], F32)
                    for l in range(lch):
                        dst = s1a[:, l, :lt] if l < LSPLIT else s1b[:, l - LSPLIT, :lt]
                        for pr in range(kch // 2):
                            nc.tensor.matmul(
                                dst,
                                w1js[j][:, 2 * pr : 2 * pr + 2],
                                xt8p[pr, 0 if l < LSPLIT else 1][:, :, l if l < LSPLIT else l - LSPLIT, :lt],
                                start=(pr == 0),
                                stop=(pr == kch // 2 - 1),
                                perf_mode=DR,
                            )
                    # two wide exps (+denominator partials via ACT accum);
                    # expA lands early (after l1's matmuls) so the s2 STTs
                    # and PSUM-bank recycling start early
                    # bf16 e: halves the DVE read stream in the STTs; the
                    # softmax weights only need ~8 mantissa bits (error
                    # averages over 2500 near-uniform terms)
                    e = epool.tile([P, lch, lt], BF16)
                    if bi == b - 1 and j == jch - 1:
                        for l in range(lch):
                            s1l = (
                                s1a[:, l, :lt] if l < LSPLIT
                                else s1b[:, l - LSPLIT, :lt]
                            )
                            nc.scalar.activation(
                                out=e[:, l], in_=s1l,
                                func=Exp, scale=1.0 / W_SCALE,
                                accum_out=dcols_all[:, bi, j, l : l + 1],
                            )
                    else:
                        nc.scalar.activation(
                            out=e[:, :LSPLIT], in_=s1a[:, :, :lt],
                            func=Exp, scale=1.0 / W_SCALE,
                            accum_out=dcols_all[:, bi, j, 0:1],
                        )
                        nc.scalar.activation(
                            out=e[:, LSPLIT:], in_=s1b[:, :, :lt],
                            func=Exp, scale=1.0 / W_SCALE,
                            accum_out=dcols_all[:, bi, j, 1:2],
                        )
                    # s2 phase: per-l matmuls through 3 rotating PSUM banks,
                    # each drained by one DVE STT (numerator partials)
                    for l in range(lch):
                        s2 = ppool2.tile([P, lt], F32)
                        for pr in range(kch // 2):
                            nc.tensor.matmul(
                                s2,
                                w2js[j][:, 2 * pr : 2 * pr + 2],
                                xt8p[pr, 0 if l < LSPLIT else 1][:, :, l if l < LSPLIT else l - LSPLIT, :lt],
                                start=(pr == 0),
                                stop=(pr == kch // 2 - 1),
                                perf_mode=DR,
                            )
                        prod = spool.tile([P, lt], F32)
                        # numer partial = sum_col e*s2/W_SCALE in one DVE pass
                        nc.vector.scalar_tensor_tensor(
                            out=prod, in0=e[:, l], scalar=1.0 / W_SCALE,
                            in1=s2, op0=mult, op1=mult,
                            accum_out=ncols_all[:, bi, j, l : l + 1],
                        )

            # single batched epilogue for all 72 (bi, j) groups
            numer = cpool.tile([P, b, jch], F32)
            denom = cpool.tile([P, b, jch], F32)
            recip = cpool.tile([P, b, jch], F32)
            nc.vector.reduce_sum(numer, ncols_all, axis=AX)
            nc.vector.reduce_sum(denom, dcols_all, axis=AX)
            nc.vector.reciprocal(recip, denom)
            nc.vector.tensor_mul(out_all, numer, recip)
            nc.vector.tensor_add(out_all, out_all, b2sb)
            nc.sync.dma_start(out=out[:], in_=out_all)

    nc.compile()
    return nc


_NC_CACHE = {}


def _get_nc():
    if "nc" not in _NC_CACHE:
        _NC_CACHE["nc"] = build_nc()
    return _NC_CACHE["nc"]


def make_in_maps(x, W1, W2, b2):
    """Host-side shard prep: pad C, pre-transpose + fp8-quantize weights,
    tanh+fp8-quantize x, fold the fp8 bias correction into a per-batch b2."""
    x = np.ascontiguousarray(np.asarray(x, dtype=np.float32))

    def prep_w(W):
        Wp = np.zeros((C_PAD, D), dtype=np.float32)
        Wp[:C] = np.asarray(W, dtype=np.float32)
        return Wp

    W1p, W2p = prep_w(W1), prep_w(W2)
    w1q8 = (W1p * W_SCALE).astype(FP8_NP)        # [C_PAD, D] fp8 payloads
    w2q8 = (W2p * W_SCALE).astype(FP8_NP)

    xt8 = np.tanh(x).astype(FP8_NP)              # [B, D, L] fp8 payloads

    # bias correction: quantized-minus-exact pipeline under uniform weights
    xtu = np.tanh(x).mean(axis=2)                # [B, D]
    m8 = xt8.astype(np.float32).mean(axis=2)
    w2q = w2q8.astype(np.float32) / W_SCALE
    corr = m8 @ w2q.T - xtu @ W2p.T              # [B, C_PAD]

    b2p = np.zeros((C_PAD,), dtype=np.float32)
    b2p[:C] = np.asarray(b2, dtype=np.float32)
    b2adj = (b2p[None, :] - corr).astype(np.float32)   # [B, C_PAD]

    # [B, D, L] -> [B, KCH//2, P, 2, LCH, LT8] with LT->LT8 zero padding
    # (d = pr*256 + kk*128 + p; device tile wants [p, kk, l, t] per pr)
    x8 = np.zeros((B, KCH // 2, P, 2, LCH, LT8), dtype=FP8_NP)
    x8[..., :LT] = xt8.reshape(B, KCH // 2, 2, P, LCH, LT).transpose(
        0, 1, 3, 2, 4, 5
    )
    in_maps = []
    for i in range(N_CORES):
        sl = slice(i * C_SH, (i + 1) * C_SH)
        def shard_w(wq8):
            A = wq8[sl].reshape(JCH, P, KCH, P)        # [j, c, k, p]
            return np.ascontiguousarray(A.transpose(0, 3, 2, 1))  # [j, p, k, c]

        w1t = shard_w(w1q8)
        w2t = shard_w(w2q8)
        b2s = np.ascontiguousarray(
            b2adj[:, sl].reshape(B, JCH, P).transpose(2, 0, 1)
        )
        in_maps.append({"x8": x8, "w1t": w1t, "w2t": w2t, "b2s": b2s})
    return in_maps


def gather_out(results):
    """results: list (per core) of {'out': [JCH, P, B]} -> full [B, C]."""
    parts = [
        np.transpose(np.asarray(r["out"], dtype=np.float32), (1, 2, 0)).reshape(B, C_SH)
        for r in results
    ]
    return np.concatenate(parts, axis=1)[:, :C]


def kernel(x, W1, W2, b2):
    nc = _get_nc()
    in_maps = make_in_maps(x, W1, W2, b2)
    res = run_bass_kernel_spmd(nc, in_maps, list(range(N_CORES)))
    return gather_out(res.results)
